# revision 16
# baseline (speedup 1.0000x reference)
"""Longformer layer + MoE on 8 trn2 cores.

Sharding: data-parallel over the 4096-token sequence (512 tokens/core,
256-token halo for the sliding-window K/V). No collectives. Attention in
fp32 (top-k gate routing must match the fp32 reference bit-for-bit at the
argmax level), MoE expert matmuls in bf16 (dense all-expert compute,
gate-weighted on-chip).
"""

import sys

for _p in ("/opt/trn_rl_repo",):
    if _p not in sys.path:
        sys.path.insert(0, _p)

import numpy as np
import ml_dtypes

# problem constants (hardcoded per contract)
S, D, H, F, E, W = 4096, 768, 12, 3072, 7, 256
NCORES = 8
T = S // NCORES          # 512 own tokens per core
TH = T + 2 * W           # 1024 halo tokens
QT = T // 128            # 4 q tiles
KW = 128 + 2 * W         # 640 key window per q tile
DT = D // 128            # 6
FT = F // 128            # 24
Dh = D // H              # 64
NH = 384                 # half of 768 free-dim split
BF16 = ml_dtypes.bfloat16

_BUILT = None
TRACE = False
LAST_RESULT = None


def _emit(nc, tc, tens):
    import concourse.bass as bass
    import concourse.mybir as mybir
    from concourse.masks import make_identity

    f32 = mybir.dt.float32
    f32r = mybir.dt.float32r
    bf16 = mybir.dt.bfloat16
    Id = mybir.ActivationFunctionType.Identity
    Exp = mybir.ActivationFunctionType.Exp
    Gelu = mybir.ActivationFunctionType.Gelu
    Sqrt = mybir.ActivationFunctionType.Sqrt
    mult = mybir.AluOpType.mult
    add = mybir.AluOpType.add
    sub = mybir.AluOpType.subtract
    is_ge = mybir.AluOpType.is_ge
    AX = mybir.AxisListType.X

    xT_d, xo_d, mask_d, wqkv_d, wo_d, bqkv_d, vecs_d, gw_d, gb_d, \
        w1_d, b1_d, w2_d, b2_d, wd_d, y_d, gwo_d = tens

    with tc.tile_pool(name="const", bufs=1) as cpool, \
         tc.tile_pool(name="vecbc", bufs=3) as vpool, \
         tc.tile_pool(name="big", bufs=3) as bigp, \
         tc.tile_pool(name="med", bufs=3) as medp, \
         tc.tile_pool(name="whalf", bufs=2) as whp, \
         tc.tile_pool(name="red", bufs=16) as redp:

        ident = cpool.tile([128, 128], f32, name="ident")
        make_identity(nc, ident)
        eps_t = cpool.tile([128, 1], f32, name="eps_t")
        nc.gpsimd.memset(eps_t, 1e-5)
        mask_sb = cpool.tile([128, QT, KW], bf16, name="mask_sb")
        nc.sync.dma_start(mask_sb, mask_d.rearrange("(t p) k -> p t k", p=128))
        bqkv_sb = cpool.tile([128, 3, DT], f32, name="bqkv_sb")
        nc.sync.dma_start(bqkv_sb, bqkv_d.rearrange("w (o p) -> p w o", p=128))
        b1_sb = cpool.tile([128, E, FT], f32, name="b1_sb")
        nc.sync.dma_start(b1_sb, b1_d.rearrange("e (o p) -> p e o", p=128))
        gatew_sb = cpool.tile([128, DT, E], f32, name="gatew_sb")
        nc.sync.dma_start(gatew_sb, gw_d.rearrange("(o p) n -> p o n", p=128))
        gb_bc = cpool.tile([128, E], f32, name="gb_bc")
        nc.gpsimd.dma_start(gb_bc, gb_d[None, :].to_broadcast((128, E)))
        gwsb = cpool.tile([128, QT, E], f32, name="gwsb")
        attnT = cpool.tile([128, DT, T], f32r, name="attnT")
        aTb = cpool.tile([128, DT, T], bf16, name="aTb")
        ao_sb = cpool.tile([128, QT, D], f32, name="ao_sb")
        moe_acc = cpool.tile([128, QT, D], f32, name="moe_acc")

        # ---------------- phase A: QKV projections (fp32) ----------------
        xT = bigp.tile([128, DT, TH], f32r, name="xT", tag="big", bufs=3)
        nc.sync.dma_start(xT, xT_d.rearrange("(o p) t -> p o t", p=128))
        kT = bigp.tile([128, DT, TH], f32r, name="kT", tag="big", bufs=3)
        v_sb = bigp.tile([128, TH // 128, D], f32r, name="v_sb", tag="big", bufs=3)
        q_sb = medp.tile([128, DT, T], f32r, name="q_sb", tag="med", bufs=1)
        bv_bc = vpool.tile([128, D], f32, name="bv_bc", tag="vec", bufs=3)
        nc.gpsimd.dma_start(bv_bc, vecs_d[0][None, :].to_broadcast((128, D)))

        with tc.tile_pool(name="psA", bufs=4, space="PSUM") as psA, \
             tc.tile_pool(name="wblk", bufs=8) as wbp:
            # q (feature-major [dout, tok]), own tokens live at halo [W, W+T)
            for dt in range(DT):
                ps = psA.tile([128, T], f32, name="psq", tag="ps")
                for kt in range(DT):
                    wb = wbp.tile([128, 128], f32r, name="wqb", tag="wblk", bufs=8)
                    nc.sync.dma_start(
                        wb, wqkv_d[0, kt * 128:(kt + 1) * 128, dt * 128:(dt + 1) * 128])
                    nc.tensor.matmul(ps, wb, xT[:, kt, W:W + T],
                                     start=(kt == 0), stop=(kt == DT - 1))
                nc.scalar.activation(q_sb[:, dt], ps, Id,
                                     bias=bqkv_sb[:, 0, dt:dt + 1], scale=0.125)
            # k (feature-major, all halo tokens)
            for dt in range(DT):
                for th in range(2):
                    ps = psA.tile([128, T], f32, name="psk", tag="ps")
                    for kt in range(DT):
                        wb = wbp.tile([128, 128], f32r, name="wkb", tag="wblk", bufs=8)
                        nc.sync.dma_start(
                            wb, wqkv_d[1, kt * 128:(kt + 1) * 128, dt * 128:(dt + 1) * 128])
                        nc.tensor.matmul(ps, wb, xT[:, kt, th * T:(th + 1) * T],
                                         start=(kt == 0), stop=(kt == DT - 1))
                    nc.scalar.activation(kT[:, dt, th * T:(th + 1) * T], ps, Id,
                                         bias=bqkv_sb[:, 1, dt:dt + 1])
            # v (token-major [tok, dout], all halo tokens)
            for nh in range(2):
                wvh = whp.tile([128, DT, NH], f32r, name="wvh", tag="whalf", bufs=2)
                nc.sync.dma_start(
                    wvh, wqkv_d[2].rearrange("(o p) d -> p o d", p=128)[:, :, nh * NH:(nh + 1) * NH])
                for tt in range(TH // 128):
                    ps = psA.tile([128, T], f32, name="psv", tag="ps")
                    for kt in range(DT):
                        nc.tensor.matmul(ps[:, :NH], xT[:, kt, tt * 128:(tt + 1) * 128],
                                         wvh[:, kt], start=(kt == 0), stop=(kt == DT - 1))
                    nc.vector.scalar_tensor_tensor(
                        v_sb[:, tt, nh * NH:(nh + 1) * NH], ps[:, :NH], 1.0,
                        bv_bc[:, nh * NH:(nh + 1) * NH], op0=mult, op1=add)

        # ------- phase B: sliding-window attention (fp32r matmuls) -------
            with tc.tile_pool(name="psS", bufs=2, space="PSUM") as psS, \
                 tc.tile_pool(name="psT", bufs=2, space="PSUM") as psT, \
                 tc.tile_pool(name="psO", bufs=2, space="PSUM") as psO, \
                 tc.tile_pool(name="pth", bufs=1) as pthp, \
                 tc.tile_pool(name="sbS", bufs=2) as sbS:
                # persistent transposed-probs buffer [k-halo-tile, q]; fringe
                # (outside any window) stays zero across heads
                PT_h = pthp.tile([128, TH // 128, T], f32r, name="PT_h")
                for t, lo, hi in ((0, 128, 512), (1, 256, 512), (2, 384, 512),
                                  (5, 0, 128), (6, 0, 256), (7, 0, 384)):
                    nc.gpsimd.memset(PT_h[:, t, lo:hi].bitcast(f32), 0.0)

                def scores_softmax(h, qt):
                    dtq, off = h // 2, 64 * (h % 2)
                    ps_s = psS.tile([128, 2, 512], f32, name="ps_s", tag="pss", bufs=2)
                    lhs_q = q_sb[off:off + 64, dtq, qt * 128:(qt + 1) * 128].bitcast(f32r)
                    nc.tensor.matmul(ps_s[:, 0, :384], lhs_q,
                                     kT[off:off + 64, dtq, qt * 128:qt * 128 + 384].bitcast(f32r),
                                     start=True, stop=True)
                    nc.tensor.matmul(ps_s[:, 1, :256], lhs_q,
                                     kT[off:off + 64, dtq, qt * 128 + 384:qt * 128 + KW].bitcast(f32r),
                                     start=True, stop=True)
                    s_sb = sbS.tile([128, KW], f32, name="s_sb", tag="ssb", bufs=2)
                    nc.vector.scalar_tensor_tensor(s_sb[:, 0:384], ps_s[:, 0, :384], 1.0,
                                                   mask_sb[:, qt, 0:384], op0=mult, op1=add)
                    nc.vector.scalar_tensor_tensor(s_sb[:, 384:KW], ps_s[:, 1, :256], 1.0,
                                                   mask_sb[:, qt, 384:KW], op0=mult, op1=add)
                    m = redp.tile([128, 1], f32, name="m", tag="red", bufs=16)
                    nc.vector.reduce_max(m, s_sb, axis=AX)
                    nm = redp.tile([128, 1], f32, name="nm", tag="red", bufs=16)
                    nc.vector.tensor_scalar_mul(nm, m, -1.0)
                    p_sb = sbS.tile([128, KW], f32, name="p_sb", tag="psb", bufs=2)
                    ssum = redp.tile([128, 1], f32, name="ssum", tag="red", bufs=16)
                    nc.scalar.activation(p_sb, s_sb, Exp, bias=nm, accum_out=ssum)
                    r = redp.tile([128, 1], f32, name="r", tag="red", bufs=16)
                    nc.vector.reciprocal(r, ssum)
                    nc.vector.tensor_scalar_mul(p_sb, p_sb, r)
                    return p_sb

                def pt_pv(h, qt, p_sb):
                    dtq, off = h // 2, 64 * (h % 2)
                    for k5 in range(5):
                        ps_t = psT.tile([128, 128], f32, name="ps_t", tag="pst", bufs=2)
                        nc.tensor.transpose(ps_t, p_sb[:, k5 * 128:(k5 + 1) * 128], ident)
                        nc.scalar.activation(PT_h[:, qt + k5, qt * 128:(qt + 1) * 128],
                                             ps_t, Id)
                    if qt == QT - 1:
                        ps_o = psO.tile([64, T], f32, name="ps_o", tag="pso", bufs=2)
                        for t in range(TH // 128):
                            nc.tensor.matmul(ps_o, v_sb[:, t, 64 * h:64 * h + 64].bitcast(f32r),
                                             PT_h[:, t].bitcast(f32r),
                                             start=(t == 0), stop=(t == TH // 128 - 1))
                        nc.scalar.activation(attnT[off:off + 64, dtq], ps_o, Id)

                iters = [(h, qt) for h in range(H) for qt in range(QT)]
                prev = None
                pending = {}
                for i, (h, qt) in enumerate(iters):
                    pending[(h, qt)] = scores_softmax(h, qt)
                    if prev is not None:
                        pt_pv(*prev, pending.pop(prev))
                    prev = (h, qt)
                pt_pv(*prev, pending.pop(prev))

        # ------------- phase C: Wo + residual + LN1 + gate/topk ----------
        ln1g = vpool.tile([128, D], f32, name="ln1g", tag="vec", bufs=3)
        nc.gpsimd.dma_start(ln1g, vecs_d[1][None, :].to_broadcast((128, D)))
        ln1b = vpool.tile([128, D], f32, name="ln1b", tag="vec", bufs=3)
        nc.gpsimd.dma_start(ln1b, vecs_d[2][None, :].to_broadcast((128, D)))
        with tc.tile_pool(name="psC", bufs=4, space="PSUM") as psC, \
             tc.tile_pool(name="psT2", bufs=2, space="PSUM") as psT2, \
             tc.tile_pool(name="psG", bufs=2, space="PSUM") as psG, \
             tc.tile_pool(name="sbC", bufs=2) as sbC:
            for nh in range(2):
                woh = whp.tile([128, DT, NH], f32r, name="woh", tag="whalf", bufs=2)
                nc.sync.dma_start(
                    woh, wo_d.rearrange("(o p) d -> p o d", p=128)[:, :, nh * NH:(nh + 1) * NH])
                for tt in range(QT):
                    ps = psC.tile([128, NH], f32, name="pswo", tag="psc")
                    for kt in range(DT):
                        nc.tensor.matmul(ps, attnT[:, kt, tt * 128:(tt + 1) * 128].bitcast(f32r),
                                         woh[:, kt].bitcast(f32r),
                                         start=(kt == 0), stop=(kt == DT - 1))
                    xo2 = sbC.tile([128, NH], f32, name="xo2", tag="xo2", bufs=6)
                    nc.sync.dma_start(
                        xo2, xo_d.rearrange("(t p) d -> p t d", p=128)[:, tt, nh * NH:(nh + 1) * NH])
                    nc.vector.scalar_tensor_tensor(ao_sb[:, tt, nh * NH:(nh + 1) * NH],
                                                   ps, 1.0, xo2, op0=mult, op1=add)
            for tt in range(QT):
                t_ap = ao_sb[:, tt]
                ssum = redp.tile([128, 1], f32, name="s1", tag="red", bufs=16)
                nc.vector.reduce_sum(ssum, t_ap, axis=AX)
                mean = redp.tile([128, 1], f32, name="mean", tag="red", bufs=16)
                nc.vector.tensor_scalar_mul(mean, ssum, 1.0 / D)
                c_sb = sbC.tile([128, D], f32, name="c_sb", tag="csb", bufs=2)
                nc.vector.tensor_scalar(c_sb, t_ap, mean, None, op0=sub)
                c2 = sbC.tile([128, D], f32, name="c2", tag="c2", bufs=2)
                vsum = redp.tile([128, 1], f32, name="vsum", tag="red", bufs=16)
                nc.scalar.activation(c2, c_sb, mybir.ActivationFunctionType.Square,
                                     accum_out=vsum)
                sd = redp.tile([128, 1], f32, name="sd", tag="red", bufs=16)
                nc.scalar.activation(sd, vsum, Sqrt, scale=1.0 / D, bias=eps_t)
                rstd = redp.tile([128, 1], f32, name="rstd", tag="red", bufs=16)
                nc.vector.reciprocal(rstd, sd)
                a_tmp = sbC.tile([128, D], f32, name="a_tmp", tag="atmp", bufs=2)
                nc.vector.scalar_tensor_tensor(a_tmp, c_sb, rstd, ln1g, op0=mult, op1=mult)
                nc.vector.tensor_tensor(a_tmp, a_tmp, ln1b, op=add)
                # transpose a_tmp -> aT blocks; gate matmul accumulates over dt
                ps_g = psG.tile([128, E], f32, name="ps_g", tag="psg")
                for dt in range(DT):
                    ps_t = psT2.tile([128, 128], f32, name="ps_t2", tag="pst2")
                    nc.tensor.transpose(ps_t, a_tmp[:, dt * 128:(dt + 1) * 128], ident)
                    t32 = sbC.tile([128, 128], f32, name="t32", tag="t32", bufs=3)
                    nc.scalar.activation(t32, ps_t, Id)
                    nc.vector.tensor_copy(aTb[:, dt, tt * 128:(tt + 1) * 128], t32)
                    nc.tensor.matmul(ps_g, t32, gatew_sb[:, dt],
                                     start=(dt == 0), stop=(dt == DT - 1))
                l_sb = sbC.tile([128, E], f32, name="l_sb", tag="lsb", bufs=2)
                nc.vector.scalar_tensor_tensor(l_sb, ps_g, 1.0, gb_bc, op0=mult, op1=add)
                # top-2 mask + softmax weights
                v1 = redp.tile([128, 1], f32, name="v1", tag="red", bufs=16)
                nc.vector.reduce_max(v1, l_sb, axis=AX)
                m1 = sbC.tile([128, E], f32, name="m1", tag="m1", bufs=2)
                nc.vector.tensor_scalar(m1, l_sb, v1, None, op0=is_ge)
                lm = sbC.tile([128, E], f32, name="lm", tag="lm", bufs=2)
                nc.vector.scalar_tensor_tensor(lm, m1, -1e9, l_sb, op0=mult, op1=add)
                v2 = redp.tile([128, 1], f32, name="v2", tag="red", bufs=16)
                nc.vector.reduce_max(v2, lm, axis=AX)
                nv1 = redp.tile([128, 1], f32, name="nv1", tag="red", bufs=16)
                nc.vector.tensor_scalar_mul(nv1, v1, -1.0)
                ed = redp.tile([128, 1], f32, name="ed", tag="red", bufs=16)
                nc.scalar.activation(ed, v2, Exp, bias=nv1)
                den = redp.tile([128, 1], f32, name="den", tag="red", bufs=16)
                nc.vector.tensor_scalar_add(den, ed, 1.0)
                rd = redp.tile([128, 1], f32, name="rd", tag="red", bufs=16)
                nc.vector.reciprocal(rd, den)
                p7 = sbC.tile([128, E], f32, name="p7", tag="p7", bufs=2)
                nc.scalar.activation(p7, l_sb, Exp, bias=nv1)
                m2 = sbC.tile([128, E], f32, name="m2", tag="m2", bufs=2)
                nc.vector.tensor_scalar(m2, l_sb, v2, None, op0=is_ge)
                nc.vector.tensor_tensor(p7, p7, m2, op=mult)
                nc.vector.tensor_scalar_mul(gwsb[:, tt], p7, rd)
                nc.sync.dma_start(gwo_d.rearrange("(t p) e -> p t e", p=128)[:, tt],
                                  gwsb[:, tt])

        # ---------------- phase D: dense MoE (bf16), gate-weighted -------
        with tc.tile_pool(name="psL1", bufs=3, space="PSUM") as psL1, \
             tc.tile_pool(name="psL2", bufs=4, space="PSUM") as psL2, \
             tc.tile_pool(name="w1p", bufs=3) as w1p, \
             tc.tile_pool(name="w2p", bufs=2) as w2p, \
             tc.tile_pool(name="sbD", bufs=2) as sbD:
            for e in range(E):
                hT = bigp.tile([128, FT, T], bf16, name="hT", tag="big", bufs=3)
                for ft in range(FT):
                    w1t = w1p.tile([128, DT, 128], bf16, name="w1t", tag="w1t", bufs=3)
                    nc.sync.dma_start(
                        w1t, w1_d[e].rearrange("(o p) f -> p o f", p=128)[:, :, ft * 128:(ft + 1) * 128])
                    ps = psL1.tile([128, T], f32, name="psl1", tag="psl1")
                    for kt in range(DT):
                        nc.tensor.matmul(ps, w1t[:, kt], aTb[:, kt],
                                         start=(kt == 0), stop=(kt == DT - 1))
                    nc.scalar.activation(hT[:, ft], ps, Gelu, bias=b1_sb[:, e, ft:ft + 1])
                b2bc = sbD.tile([128, D], f32, name="b2bc", tag="b2bc", bufs=2)
                nc.gpsimd.dma_start(b2bc, b2_d[e][None, :].to_broadcast((128, D)))
                for nh in range(2):
                    w2h = w2p.tile([128, FT, NH], bf16, name="w2h", tag="w2h", bufs=2)
                    nc.sync.dma_start(
                        w2h, w2_d[e].rearrange("(o p) d -> p o d", p=128)[:, :, nh * NH:(nh + 1) * NH])
                    for tt in range(QT):
                        ps = psL2.tile([128, NH], f32, name="psl2", tag="psl2")
                        for ft in range(FT):
                            nc.tensor.matmul(ps, hT[:, ft, tt * 128:(tt + 1) * 128],
                                             w2h[:, ft], start=(ft == 0), stop=(ft == FT - 1))
                        eo = sbD.tile([128, NH], f32, name="eo", tag="eo", bufs=3)
                        nc.vector.scalar_tensor_tensor(eo, ps, 1.0,
                                                       b2bc[:, nh * NH:(nh + 1) * NH],
                                                       op0=mult, op1=add)
                        dst = moe_acc[:, tt, nh * NH:(nh + 1) * NH]
                        gwe = gwsb[:, tt, e:e + 1]
                        if e == 0:
                            nc.vector.tensor_scalar_mul(dst, eo, gwe)
                        else:
                            nc.vector.scalar_tensor_tensor(dst, eo, gwe, dst,
                                                           op0=mult, op1=add)

        # ---------------- phase E: Wd + residual + LN2 + out -------------
        ln2g = vpool.tile([128, D], f32, name="ln2g", tag="vec", bufs=3)
        nc.gpsimd.dma_start(ln2g, vecs_d[3][None, :].to_broadcast((128, D)))
        ln2b = vpool.tile([128, D], f32, name="ln2b", tag="vec", bufs=3)
        nc.gpsimd.dma_start(ln2b, vecs_d[4][None, :].to_broadcast((128, D)))
        bd_bc = vpool.tile([128, D], f32, name="bd_bc", tag="vec", bufs=3)
        nc.gpsimd.dma_start(bd_bc, vecs_d[5][None, :].to_broadcast((128, D)))
        moeT = cpool.tile([128, DT, T], bf16, name="moeT")
        with tc.tile_pool(name="psT3", bufs=2, space="PSUM") as psT3, \
             tc.tile_pool(name="psE", bufs=2, space="PSUM") as psE, \
             tc.tile_pool(name="sbE", bufs=2) as sbE:
            for tt in range(QT):
                for dt in range(DT):
                    ps_t = psT3.tile([128, 128], f32, name="ps_t3", tag="pst3")
                    nc.tensor.transpose(ps_t, moe_acc[:, tt, dt * 128:(dt + 1) * 128], ident)
                    nc.scalar.activation(moeT[:, dt, tt * 128:(tt + 1) * 128], ps_t, Id)
            for nh in range(2):
                wdh = whp.tile([128, DT, NH], bf16, name="wdh", tag="whalf", bufs=2)
                nc.sync.dma_start(
                    wdh, wd_d.rearrange("(o p) d -> p o d", p=128)[:, :, nh * NH:(nh + 1) * NH])
                for tt in range(QT):
                    ps = psE.tile([128, NH], f32, name="pswd", tag="pse")
                    for kt in range(DT):
                        nc.tensor.matmul(ps, moeT[:, kt, tt * 128:(tt + 1) * 128],
                                         wdh[:, kt], start=(kt == 0), stop=(kt == DT - 1))
                    f_half = sbE.tile([128, NH], f32, name="f_half", tag="fh", bufs=4)
                    nc.vector.scalar_tensor_tensor(
                        f_half, ps, 1.0, ao_sb[:, tt, nh * NH:(nh + 1) * NH],
                        op0=mult, op1=add)
                    nc.vector.tensor_tensor(
                        ao_sb[:, tt, nh * NH:(nh + 1) * NH], f_half,
                        bd_bc[:, nh * NH:(nh + 1) * NH], op=add)
            # LN2 (ao_sb now holds ffn + attention_output + bd)
            for tt in range(QT):
                t_ap = ao_sb[:, tt]
                ssum = redp.tile([128, 1], f32, name="s2", tag="red", bufs=16)
                nc.vector.reduce_sum(ssum, t_ap, axis=AX)
                mean = redp.tile([128, 1], f32, name="mean2", tag="red", bufs=16)
                nc.vector.tensor_scalar_mul(mean, ssum, 1.0 / D)
                c_sb = sbE.tile([128, D], f32, name="c_sb2", tag="csb2", bufs=2)
                nc.vector.tensor_scalar(c_sb, t_ap, mean, None, op0=sub)
                c2 = sbE.tile([128, D], f32, name="c22", tag="c22", bufs=2)
                vsum = redp.tile([128, 1], f32, name="vsum2", tag="red", bufs=16)
                nc.scalar.activation(c2, c_sb, mybir.ActivationFunctionType.Square,
                                     accum_out=vsum)
                sd = redp.tile([128, 1], f32, name="sd2", tag="red", bufs=16)
                nc.scalar.activation(sd, vsum, Sqrt, scale=1.0 / D, bias=eps_t)
                rstd = redp.tile([128, 1], f32, name="rstd2", tag="red", bufs=16)
                nc.vector.reciprocal(rstd, sd)
                y_tmp = sbE.tile([128, D], f32, name="y_tmp", tag="ytmp", bufs=2)
                nc.vector.scalar_tensor_tensor(y_tmp, c_sb, rstd, ln2g, op0=mult, op1=mult)
                nc.vector.tensor_tensor(y_tmp, y_tmp, ln2b, op=add)
                nc.sync.dma_start(y_d.rearrange("(t p) d -> p t d", p=128)[:, tt], y_tmp)


def _build():
    global _BUILT
    if _BUILT is not None:
        return _BUILT
    import concourse.mybir as mybir
    import concourse.tile as tile
    from concourse import bacc

    f32 = mybir.dt.float32
    bf16 = mybir.dt.bfloat16
    nc = bacc.Bacc("TRN2", target_bir_lowering=False, debug=False,
                   num_devices=NCORES)
    tens = (
        nc.dram_tensor("xT_halo", [D, TH], mybir.dt.float32r, kind="ExternalInput"),
        nc.dram_tensor("x_own", [T, D], f32, kind="ExternalInput"),
        nc.dram_tensor("mask", [T, KW], bf16, kind="ExternalInput"),
        nc.dram_tensor("wqkv", [3, D, D], mybir.dt.float32r, kind="ExternalInput"),
        nc.dram_tensor("wo", [D, D], mybir.dt.float32r, kind="ExternalInput"),
        nc.dram_tensor("bqkv", [3, D], f32, kind="ExternalInput"),
        nc.dram_tensor("vecs", [6, D], f32, kind="ExternalInput"),
        nc.dram_tensor("gate_w", [D, E], f32, kind="ExternalInput"),
        nc.dram_tensor("gate_b", [E], f32, kind="ExternalInput"),
        nc.dram_tensor("w1", [E, D, F], bf16, kind="ExternalInput"),
        nc.dram_tensor("b1", [E, F], f32, kind="ExternalInput"),
        nc.dram_tensor("w2", [E, F, D], bf16, kind="ExternalInput"),
        nc.dram_tensor("b2", [E, D], f32, kind="ExternalInput"),
        nc.dram_tensor("wd", [D, D], bf16, kind="ExternalInput"),
        nc.dram_tensor("y", [T, D], f32, kind="ExternalOutput"),
        nc.dram_tensor("gw_out", [T, E], f32, kind="ExternalOutput"),
    )
    with tile.TileContext(nc) as tc:
        _emit(nc, tc, tens)
    nc.compile()
    _BUILT = nc
    return nc


def kernel(**inputs):
    from concourse.bass_utils import run_bass_kernel_spmd

    f = lambda k: np.asarray(inputs[k], dtype=np.float32)
    x = f("hidden_states")[0]                      # [S, D]
    wqkv = np.ascontiguousarray(np.stack([f("Wq"), f("Wk"), f("Wv")]))
    bqkv = np.ascontiguousarray(
        np.stack([f("bq") * 0.125, f("bk"), f("bv")]))
    vecs = np.ascontiguousarray(np.stack(
        [f("bv"), f("ln1_g"), f("ln1_b"), f("ln2_g"), f("ln2_b"), f("bd")]))
    shared = {
        "wqkv": wqkv, "wo": np.ascontiguousarray(f("Wo")), "bqkv": bqkv,
        "vecs": vecs,
        "gate_w": np.ascontiguousarray(f("gate_W")),
        "gate_b": np.ascontiguousarray(f("gate_b")),
        "w1": np.ascontiguousarray(f("W1e").astype(BF16)),
        "b1": np.ascontiguousarray(f("b1e")),
        "w2": np.ascontiguousarray(f("W2e").astype(BF16)),
        "b2": np.ascontiguousarray(f("b2e")),
        "wd": np.ascontiguousarray(f("Wd").astype(BF16)),
    }
    bo = f("bo")
    in_maps = []
    for c in range(NCORES):
        t0 = c * T
        halo = np.zeros((TH, D), np.float32)
        lo, hi = t0 - W, t0 + T + W
        slo, shi = max(lo, 0), min(hi, S)
        halo[slo - lo:shi - lo] = x[slo:shi]
        # additive score mask: query i (local), key j in window
        i = np.arange(128)[:, None]
        j = np.arange(KW)[None, :]
        m = np.zeros((T, KW), np.float32)
        for qt in range(QT):
            qpos = t0 + qt * 128 + i
            kpos = t0 + qt * 128 - W + j
            valid = (np.abs(kpos - qpos) <= W) & (kpos >= 0) & (kpos < S)
            m[qt * 128:(qt + 1) * 128] = np.where(valid, 0.0, -1e9)
        im = {
            "xT_halo": np.ascontiguousarray(halo.T),
            "x_own": np.ascontiguousarray(x[t0:t0 + T] + bo[None, :]),
            "mask": np.ascontiguousarray(m.astype(BF16)),
        }
        im.update(shared)
        in_maps.append(im)

    nc = _build()
    kw = {}
    if TRACE:
        kw = dict(trace=True, trace_cores=list(range(NCORES)), stitch_traces=False)
    res = run_bass_kernel_spmd(nc, in_maps, core_ids=list(range(NCORES)), **kw)
    global LAST_RESULT
    LAST_RESULT = res
    y = np.concatenate([res.results[c]["y"] for c in range(NCORES)], axis=0)
    gw = np.concatenate([res.results[c]["gw_out"] for c in range(NCORES)], axis=0)
    return y.reshape(1, S, D), gw.reshape(1, S, E)
